# revision 4
# baseline (speedup 1.0000x reference)
"""Single-head causal attention on 8 Trainium2 NeuronCores.

Problem: B=8, T=2048, C=1024, H=128 (fp32).
    q = x@Wq; k = x@Wk; v = x@Wv
    out = softmax(causal(q k^T / sqrt(H))) @ v

Sharding: data-parallel over batch — core b computes batch element b.

Per-core kernel (matmuls in fp32r, which streams at 1 cyc/row for
free-dim >= 256 vs 4 cyc/row for plain fp32):
  - x is fed pre-transposed from the host as xT [C, T] so the
    contraction dim C lands on SBUF partitions directly.
  - qT, kT, vT [H=128, T] = W^T @ xT   (H on partitions)
  - V [s, H] via PE transpose of vT (needed as matmul lhsT for PV)
  - per 512-wide t-chunk j, per pair of 128-wide s-blocks (i0,i1):
      S^T [s, t] = kT_i^T @ qT_j   (two matmuls into one 2-bank tile)
      diagonal pairs: += additive causal mask (DVE)
      P = exp(scale * S^T)         (one ScalarE op per pair, PSUM->SBUF)
      outT_j  += V_i^T @ P_i       (PSUM accumulate)
      rowsum_j += ones^T @ P_i     (PSUM accumulate, M=1)
  - outputs: unnormalized outT [128, T] and rowsum [1, T];
    the host divides and transposes (B*T*H fp32 divides, trivial).

Start-up latency hiding: xT chunk 0 is DMA'd in eight 256KB pieces so
the first projection matmul can start ~3us in; dummy PE transposes
warm the HAM clock gate during the DMA head.
"""

import numpy as np

import concourse.bass as bass
import concourse.tile as tile
from concourse import bacc, mybir
from concourse.bass_utils import run_bass_kernel_spmd
from concourse.masks import make_identity

B, T, C, H = 8, 2048, 1024, 128
N_CORES = 8
TCH = 512                # t-chunk width
N_TCH = T // TCH         # 4
SB = 128                 # s-block width
N_SB = T // SB           # 16
KCH = C // 128           # 8 contraction chunks
SCALE = float(H) ** -0.5
MASK_VAL = -1e30
N_WARMUP = 40            # dummy PE transposes to warm the clock gate

F32 = mybir.dt.float32
F32R = mybir.dt.float32r


def build_graph():
    nc = bacc.Bacc("TRN2", target_bir_lowering=False, debug=False,
                   num_devices=N_CORES)

    xT_d = nc.dram_tensor("xT", [C, T], F32R, kind="ExternalInput").ap()
    wq_d = nc.dram_tensor("Wq", [C, H], F32R, kind="ExternalInput").ap()
    wk_d = nc.dram_tensor("Wk", [C, H], F32R, kind="ExternalInput").ap()
    wv_d = nc.dram_tensor("Wv", [C, H], F32R, kind="ExternalInput").ap()
    outT_d = nc.dram_tensor("outT", [H, T], F32, kind="ExternalOutput").ap()
    rowsum_d = nc.dram_tensor("rowsum", [1, T], F32, kind="ExternalOutput").ap()

    xT_r = xT_d.rearrange("(k p) t -> p k t", p=128)

    with tile.TileContext(nc) as tc:
        with (
            tc.tile_pool(name="const", bufs=1) as cpool,
            tc.tile_pool(name="sb", bufs=1) as sbpool,
            tc.tile_pool(name="pp", bufs=2, space="PSUM") as pp_pool,
            tc.tile_pool(name="ps", bufs=2, space="PSUM") as ps_pool,
            tc.tile_pool(name="pacc", bufs=1, space="PSUM") as pacc_pool,
            tc.tile_pool(name="prow", bufs=1, space="PSUM") as prow_pool,
            tc.tile_pool(name="pt", bufs=3) as p_pool,
        ):
            # ---- constants -------------------------------------------------
            ident = cpool.tile([128, 128], F32, tag="ident")
            make_identity(nc, ident[:])

            ones_f = cpool.tile([128, 1], F32, tag="ones_f")
            nc.gpsimd.memset(ones_f[:], 1.0)
            ones = cpool.tile([128, 1], F32R, tag="ones")
            nc.vector.tensor_copy(ones[:], ones_f[:])

            # Additive causal masks for the diagonal s-block pairs.
            # masksP[:, dp, u*512:(u+1)*512] masks s-block r = 2*dp + u of
            # the diagonal group: t_local - 128r - s_local >= 0 -> keep.
            masksP = cpool.tile([128, 2, 2 * TCH], F32, tag="masks")
            nc.gpsimd.memset(masksP[:], 0.0)
            for rr in range(4):
                nc.gpsimd.affine_select(
                    out=masksP[:, rr // 2, (rr % 2) * TCH:(rr % 2 + 1) * TCH],
                    in_=masksP[:, rr // 2, (rr % 2) * TCH:(rr % 2 + 1) * TCH],
                    compare_op=mybir.AluOpType.is_ge,
                    fill=MASK_VAL,
                    base=-128 * rr,
                    pattern=[[1, TCH]],
                    channel_multiplier=-1,
                )

            # ---- DMAs: Wq first, then chunk-0 xT pieces, then the rest ----
            wq = cpool.tile([128, KCH, H], F32R, tag="wq")
            nc.sync.dma_start(wq[:], wq_d.rearrange("(k p) h -> p k h", p=128))

            xT0 = []
            for k in range(KCH):
                t_ = sbpool.tile([128, TCH], F32R, tag=f"xT0_{k}")
                nc.sync.dma_start(t_[:], xT_r[:, k, 0:TCH])
                xT0.append(t_)

            wk_t = cpool.tile([128, KCH, H], F32R, tag="wk")
            nc.sync.dma_start(wk_t[:], wk_d.rearrange("(k p) h -> p k h", p=128))
            wv_t = cpool.tile([128, KCH, H], F32R, tag="wv")
            nc.sync.dma_start(wv_t[:], wv_d.rearrange("(k p) h -> p k h", p=128))
            w_sb = [wq, wk_t, wv_t]

            xTj = [None] * N_TCH
            for j in range(1, N_TCH):
                t_ = sbpool.tile([128, KCH, TCH], F32R, tag=f"xT_{j}")
                nc.sync.dma_start(t_[:], xT_r[:, :, j * TCH:(j + 1) * TCH])
                xTj[j] = t_

            def xpiece(j, k):
                return xT0[k][:] if j == 0 else xTj[j][:, k, :]

            # ---- PE warm-up during the DMA head ---------------------------
            warm = pp_pool.tile([128, 128], F32, tag="pp")
            for _ in range(N_WARMUP):
                nc.tensor.transpose(warm[:], ident[:], ident[:])
            warm_out = cpool.tile([128, 1], F32, tag="warm_out")
            nc.vector.tensor_copy(warm_out[:], warm[:, 0:1])

            qT = sbpool.tile([128, T], F32R, tag="qT")
            kT = sbpool.tile([128, T], F32R, tag="kT")
            vT = sbpool.tile([128, T], F32, tag="vT")
            V = sbpool.tile([128, N_SB, H], F32R, tag="V")
            outT_sb = sbpool.tile([128, T], F32, tag="outT")
            rowsum_sb = sbpool.tile([1, T], F32, tag="rowsum")

            for j in range(N_TCH):
                tsl = slice(j * TCH, (j + 1) * TCH)

                # ---- projections for this t-chunk --------------------------
                for w, dst in ((w_sb[0], qT), (w_sb[1], kT), (w_sb[2], vT)):
                    ps = pp_pool.tile([128, TCH], F32, tag="pp")
                    for k in range(KCH):
                        nc.tensor.matmul(
                            ps[:],
                            w[:, k, :],
                            xpiece(j, k),
                            start=(k == 0),
                            stop=(k == KCH - 1),
                        )
                    nc.vector.tensor_copy(dst[:, tsl], ps[:])

                # ---- V blocks for this t-chunk (transpose vT) --------------
                pt = pp_pool.tile([128, TCH], F32, tag="pp")
                for q in range(4):
                    sb = 4 * j + q
                    nc.tensor.transpose(
                        pt[:, q * 128:(q + 1) * 128],
                        vT[:, sb * 128:(sb + 1) * 128],
                        ident[:],
                    )
                nc.vector.tensor_copy(V[:, 4 * j:4 * (j + 1), :], pt[:])

                # ---- attention for this t-chunk, s-blocks in pairs ---------
                n_i = 4 * j + 4
                n_pairs = n_i // 2
                acc = pacc_pool.tile([128, TCH], F32, tag="acc")
                rs = prow_pool.tile([1, TCH], F32, tag="rs")

                P_tiles = {}

                def issue_pair(m, j=j, tsl=tsl, P_tiles=P_tiles):
                    Sp = ps_pool.tile([128, 2 * TCH], F32, tag="S")
                    for h in range(2):
                        i = 2 * m + h
                        nc.tensor.matmul(
                            Sp[:, h * TCH:(h + 1) * TCH],
                            kT[:, i * SB:(i + 1) * SB],
                            qT[:, tsl],
                            start=True,
                            stop=True,
                        )
                    if 2 * m >= 4 * j:
                        dp = (2 * m - 4 * j) // 2
                        nc.vector.tensor_add(Sp[:], Sp[:], masksP[:, dp, :])
                    P = p_pool.tile([128, 2 * TCH], F32R, tag="P")
                    nc.scalar.activation(
                        P[:], Sp[:], mybir.ActivationFunctionType.Exp,
                        scale=SCALE,
                    )
                    P_tiles[m] = P

                for m in range(n_pairs):
                    if m == 0:
                        issue_pair(0)
                        if n_pairs > 1:
                            issue_pair(1)
                    elif m + 1 < n_pairs:
                        issue_pair(m + 1)
                    P = P_tiles.pop(m)
                    for h in range(2):
                        i = 2 * m + h
                        nc.tensor.matmul(
                            acc[:], V[:, i, :], P[:, h * TCH:(h + 1) * TCH],
                            start=(i == 0), stop=(i == n_i - 1),
                        )
                        nc.tensor.matmul(
                            rs[:], ones[:], P[:, h * TCH:(h + 1) * TCH],
                            start=(i == 0), stop=(i == n_i - 1),
                        )

                nc.vector.tensor_copy(outT_sb[:, tsl], acc[:])
                nc.vector.tensor_copy(rowsum_sb[:, tsl], rs[:])
                nc.sync.dma_start(outT_d[:, tsl], outT_sb[:, tsl])
                nc.sync.dma_start(rowsum_d[:, tsl], rowsum_sb[:, tsl])

    nc.compile()
    return nc


_CACHE = {}


def _get_graph():
    if "nc" not in _CACHE:
        _CACHE["nc"] = build_graph()
    return _CACHE["nc"]


def kernel(x, Wq, Wk, Wv):
    nc = _get_graph()
    x = np.asarray(x, dtype=np.float32)
    wq = np.ascontiguousarray(np.asarray(Wq, dtype=np.float32))
    wk = np.ascontiguousarray(np.asarray(Wk, dtype=np.float32))
    wv = np.ascontiguousarray(np.asarray(Wv, dtype=np.float32))

    in_maps = []
    for b in range(B):
        in_maps.append({
            "xT": np.ascontiguousarray(x[b].T),
            "Wq": wq, "Wk": wk, "Wv": wv,
        })

    res = run_bass_kernel_spmd(nc, in_maps, list(range(N_CORES)))

    outs = np.empty((B, T, H), dtype=np.float32)
    for b in range(B):
        oT = res.results[b]["outT"]          # [H, T]
        rsum = res.results[b]["rowsum"]      # [1, T]
        outs[b] = (oT / rsum).T
    return outs
